# revision 50
# baseline (speedup 1.0000x reference)
"""BiLSTM + prototype-distance kernel for 8 trn2 NeuronCores.

Sharding: 8 cores = 4 batch-shards (8 rows each) x 2 directions.
Backward-direction cores receive time-reversed input ids and run the
identical SPMD program. Host combines per-core partial outputs:
    out = 4*(xp_f + xp_b) - 4*(x2_f + x2_b) - ||protos||^2
(xp/x2 are computed from h/2 on device; the 4x compensates).

Numerics / dataflow:
- Both GEMMs (xg = e @ w_ih and the recurrent h @ w_hh) run as fp8e4
  DoubleRow matmuls (2 K-tiles per instruction, 0.5 cycles/row).
  fp8 DoubleRow can only write PSUM partition 0, so the [128,512]
  gate layout (partition = 4 hidden-chunks x 32 rows) is produced via
  a block-diagonal stationary operand: the contraction is extended to
  K=(chunk,u)=2048 and h (resp. embT) is laid out block-diagonally
  (zeros are exact in fp8). 8 accumulating DoubleRow matmuls then
  yield all four 32-partition chunk blocks in one PSUM tile.
- Quantization scales: h/2 x64, w_hh x(2*64), embT x8, w_ih x64; the
  xg path is compensated in the bias add (x 1/512), the recurrent
  path by injecting xg through a 0/4096 selector matmul and applying
  sigmoid(G/4096).
- tanh(x) = 2*sigmoid(2x)-1 throughout, so the scalar engine never
  swaps activation tables.
"""

import sys
import numpy as np

sys.path.insert(0, "/opt/trn_rl_repo")

import concourse.bass as bass  # noqa: E402
import concourse.tile as tile  # noqa: E402
import concourse.mybir as mybir  # noqa: E402
from concourse import bacc  # noqa: E402
from concourse.bass_utils import run_bass_kernel_spmd  # noqa: E402

F32 = mybir.dt.float32
F16 = mybir.dt.float16
BF16 = mybir.dt.bfloat16
FP8 = mybir.dt.float8e4
I32 = mybir.dt.int32
DR = mybir.MatmulPerfMode.DoubleRow
ALU = mybir.AluOpType

V, E, HD, P = 50000, 512, 1024, 128
H2 = HD // 2          # 512 per-direction hidden
B, T = 32, 512
BS = 8                # batch rows per core
NG = T // 4           # granules (4 timesteps each)
GMAP = [2, 0, 1, 3]   # our gate order (g, i, f, o) -> pytorch row-block order
S_W = 64.0            # fp8 scale on w_hh / w_ih
S_H = 64.0            # fp8 scale on h/2
S_E = 8.0             # fp8 scale on embeddings
SEL = S_W * S_H       # selector magnitude; sigmoid scale = 1/SEL
DISABLE = set()       # debug: subsystem names to strip from the program


def _arrange_w(w, scale_g):
    """w: (2048, K) -> (4, 128, 2048) tiles: arr[k][kk, 512c+128g+j] =
    w[512*GMAP[g] + 128c + j, 128k + kk] (*2 on the tanh gate)."""
    K = w.shape[1]
    w4 = w.reshape(4, H2, K)[GMAP].copy()      # (gamma, 512, K)
    if scale_g:
        w4[0] *= 2.0
    # -> [gamma, c, j, k, kk]
    w5 = w4.reshape(4, 4, 128, K // 128, 128)
    # arr[k, kk, c, gamma, j]
    arr = np.transpose(w5, (3, 4, 1, 0, 2)).reshape(K // 128, 128, 2048)
    return np.ascontiguousarray(arr, dtype=np.float32)


def _bd_w(arr):
    """arr: (4, 128, 2048) -> block-diag rhs (128, 8, 2, 512):
    bd[kk, kp, i, n] = arr[(2kp+i)%4][kk, 512*((2kp+i)//4) + n]."""
    bd = np.empty((128, 16, 512), np.float32)
    for kb in range(16):
        bd[:, kb, :] = arr[kb % 4][:, 512 * (kb // 4):512 * (kb // 4) + 512]
    return np.ascontiguousarray(bd.reshape(128, 8, 2, 512))


def _arrange_b(b_total):
    b4 = b_total.reshape(4, H2)[GMAP].copy()
    b4[0] *= 2.0
    # b_arr[512c + 128gamma + j] = b4[gamma, 128c + j]
    arr = np.transpose(b4.reshape(4, 4, 128), (1, 0, 2)).reshape(4, 512)
    bb = np.zeros((128, 512), np.float32)
    for c in range(4):
        bb[32 * c:32 * c + 32, :] = arr[c][None, :]
    return bb


def _make_sel4():
    """(128, 4*128): sel[p, 128*tt + q] = SEL iff q and p in the same
    32-block and p%32 == q%32 + 8*tt  (xg row selector, one matmul)."""
    sel = np.zeros((128, 4, 128), np.float32)
    for tt in range(4):
        for c in range(4):
            for m in range(32 - 8 * tt):
                sel[32 * c + 8 * tt + m, tt, 32 * c + m] = SEL
    return sel.reshape(128, 512)


def _arrange_idx(ids_shard, n_gran):
    """ids_shard: (8, T) -> (32, n_gran) int32: [8*tt + b, g] = ids[b, 4g+tt]."""
    idx = np.zeros((32, n_gran), np.int32)
    for g in range(n_gran):
        for tt in range(4):
            for b in range(BS):
                idx[8 * tt + b, g] = ids_shard[b, 4 * g + tt]
    return idx


def build_program(n_gran=NG):
    """Build the SPMD program (one core's view)."""
    nc = bacc.Bacc("TRN2", target_bir_lowering=False, debug=False)

    emb = nc.dram_tensor("emb", [V, E], BF16, kind="ExternalInput").ap()
    idx_d = nc.dram_tensor("idx", [32, n_gran], I32, kind="ExternalInput").ap()
    wih_d = nc.dram_tensor("wih", [128, 8, 2, 512], FP8, kind="ExternalInput").ap()
    whh_d = nc.dram_tensor("whh", [128, 8, 2, 512], FP8, kind="ExternalInput").ap()
    bb_d = nc.dram_tensor("bb", [128, 512], F32, kind="ExternalInput").ap()
    pt_d = nc.dram_tensor("pt", [4, 128, 128], BF16, kind="ExternalInput").ap()
    sel_d = nc.dram_tensor("sel", [128, 512], BF16, kind="ExternalInput").ap()

    xp_d = nc.dram_tensor("xp", [128, n_gran * 128], F32, kind="ExternalOutput").ap()
    x2_d = nc.dram_tensor("x2", [128, 4 * n_gran], F32, kind="ExternalOutput").ap()

    with tile.TileContext(nc) as tc:
        _body(tc, n_gran, emb, idx_d, wih_d, whh_d, bb_d, pt_d, sel_d,
              xp_d, x2_d)

    nc.compile()
    return nc


def _body(tc, n_gran, emb, idx_d, wih_d, whh_d, bb_d, pt_d, sel_d, xp_d, x2_d):
    nc = tc.nc
    from contextlib import ExitStack
    ctx = ExitStack()
    const = ctx.enter_context(tc.tile_pool(name="const", bufs=1))
    state = ctx.enter_context(tc.tile_pool(name="state", bufs=1))
    work = ctx.enter_context(tc.tile_pool(name="work", bufs=2))
    psum_g = ctx.enter_context(tc.tile_pool(name="psg", bufs=2, space="PSUM"))
    psum_m = ctx.enter_context(tc.tile_pool(name="psm", bufs=2, space="PSUM"))
    psum_t = ctx.enter_context(tc.tile_pool(name="pst", bufs=1, space="PSUM"))
    psum_h = ctx.enter_context(tc.tile_pool(name="psh", bufs=1, space="PSUM"))
    psum_p = ctx.enter_context(tc.tile_pool(name="psp", bufs=1, space="PSUM"))
    psum_w = ctx.enter_context(tc.tile_pool(name="psw", bufs=1, space="PSUM"))

    # ---- resident tensors -------------------------------------------------
    wih = const.tile([128, 8, 2, 512], FP8)
    whh = const.tile([128, 8, 2, 512], FP8)
    bb = const.tile([128, 512], F32)
    pt = const.tile([128, 4 * 128], BF16)
    sel = const.tile([128, 512], BF16)
    idx = const.tile([32, n_gran], I32)
    ident = const.tile([128, 128], BF16)

    nc.sync.dma_start(wih[:], wih_d[:])
    nc.sync.dma_start(whh[:], whh_d[:])
    for k in range(4):
        nc.sync.dma_start(pt[:, 128 * k:128 * (k + 1)], pt_d[k])
    nc.sync.dma_start(bb[:], bb_d[:])
    nc.sync.dma_start(sel[:], sel_d[:])
    nc.sync.dma_start(idx[:], idx_d[:])

    from concourse.masks import make_identity
    make_identity(nc, ident[:])

    # state
    c_st = state.tile([128, 128], F16)
    # block-diagonal fp8 stationary tiles: hbd[a][kk, b, 32a+m] = S_H*h/2
    hbd = [state.tile([128, 4, 128], FP8, name=f"hbd{a}") for a in range(4)]
    ebd = [state.tile([128, 4, 128], FP8, name=f"ebd{a}") for a in range(4)]
    hist = state.tile([128, 4, 4, 32], BF16)     # [j, c-chunk, tt, m] = h/2
    emb_ring = state.tile([32, 4 * 512], BF16)   # slot = g%4
    xg_ring = state.tile([128, 4 * 512], BF16)   # slot = g%4
    x2buf = state.tile([128, 4 * n_gran], F32)
    sq = state.tile([128, 128], F16)

    nc.gpsimd.memset(c_st[:], 0.0)
    for a in range(4):
        nc.gpsimd.memset(hbd[a][:], 0.0)
        nc.gpsimd.memset(ebd[a][:], 0.0)
    nc.gpsimd.memset(hist[:], 0.0)
    nc.gpsimd.memset(x2buf[:], 0.0)
    nc.gpsimd.memset(xg_ring[:], 0.0)
    nc.gpsimd.memset(emb_ring[:], 0.0)
    nc.gpsimd.memset(sq[:], 0.0)

    def gather(g):
        s = 512 * (g % 4)
        nc.gpsimd.indirect_dma_start(
            out=emb_ring[:, s:s + 512],
            out_offset=None,
            in_=emb[:],
            in_offset=bass.IndirectOffsetOnAxis(ap=idx[:, g:g + 1], axis=0),
        )

    def bd_fill_gp(dst, src_sb):
        """Write the 4 block-diagonal copies dst[a][:, b, 32a+m] =
        src[:, 32b+m] from an SBUF fp8 staging tile (gpsimd copies)."""
        for a in range(4):
            nc.gpsimd.affine_select(
                out=dst[a][:, :, 32 * a:32 * a + 32], in_=src_sb[:],
                pattern=[[0, 4], [0, 32]], compare_op=ALU.is_equal,
                fill=0.0, base=0, channel_multiplier=0)

    def bd_fill_vec(dst, src_psum, scale):
        """Same, directly from PSUM on the vector engine (with scale)."""
        for a in range(4):
            nc.vector.scalar_tensor_tensor(
                out=dst[a][:, :, 32 * a:32 * a + 32], in0=src_psum[:],
                scalar=scale, in1=ident[:], op0=ALU.mult, op1=ALU.bypass)

    # phase1 split into per-step pieces to avoid head-of-line blocking of
    # the serial recurrence in the engine queues.
    def phase1_piece(g, piece):
        s = 512 * (g % 4)
        if piece == 0:
            tp = psum_t.tile([128, 128], BF16)
            for k in range(4):
                nc.tensor.matmul(
                    tp[:, 32 * k:32 * k + 32],
                    lhsT=emb_ring[:, s + 128 * k:s + 128 * (k + 1)],
                    rhs=ident[:32, :32],
                    is_transpose=True, start=(k == 0), stop=(k == 3))
            tq = work.tile([128, 128], FP8, tag="tq")
            nc.vector.scalar_tensor_tensor(
                out=tq[:], in0=tp[:], scalar=S_E, in1=ident[:],
                op0=ALU.mult, op1=ALU.bypass)
            bd_fill_gp(ebd, tq)
        else:
            if piece == 1:
                phase1_piece.mm = psum_m.tile([128, 512], F32)
            mm = phase1_piece.mm
            for kp in range([0, 0, 2, 5][piece], [0, 2, 5, 8][piece]):
                a, q = kp // 2, kp % 2
                nc.tensor.matmul(
                    mm[:], lhsT=ebd[a][:, 2 * q:2 * q + 2, :],
                    rhs=wih[:, kp], start=(kp == 0), stop=(kp == 7),
                    perf_mode=DR)
            if piece == 3:
                for half in range(2):
                    sl = slice(s + 256 * half, s + 256 * (half + 1))
                    nc.vector.scalar_tensor_tensor(
                        out=xg_ring[:, sl],
                        in0=mm[:, 256 * half:256 * (half + 1)],
                        scalar=1.0 / (S_E * S_W),
                        in1=bb[:, 256 * half:256 * (half + 1)],
                        op0=ALU.mult, op1=ALU.add)

    warm_tile = [None]

    def warm(rhs, n=1):
        """Dummy matmul into a scratch PSUM bank: keeps the PE power-state
        ramped through the postproc gap (result unused). Passing a
        mid-chain tensor as rhs delays execution into the gap."""
        if warm_tile[0] is None:
            warm_tile[0] = psum_w.tile([128, 512], F32, name="warmt")
        w = rhs.free_size()
        for _ in range(n):
            nc.tensor.matmul(
                warm_tile[0][:, 0:w], lhsT=sel[:, 0:128], rhs=rhs,
                start=True, stop=True, skip_group_check=True)

    def emit_sel(t):
        """Preload the xg injection for step t into a fresh PSUM buffer:
        one K=128 matmul, sel = 0/4096 shift matrix."""
        tt, slot = t % 4, 512 * ((t // 4) % 4)
        G = psum_g.tile([128, 512], F32)
        nc.tensor.matmul(
            G[:], lhsT=sel[:, 128 * tt:128 * (tt + 1)],
            rhs=xg_ring[:, slot:slot + 512],
            start=True, stop=False)
        return G

    def step_mm(G):
        # recurrent part: 8 fp8 DoubleRow matmuls, block-diag stationary
        for kp in range(8):
            a, q = kp // 2, kp % 2
            nc.tensor.matmul(
                G[:], lhsT=hbd[a][:, 2 * q:2 * q + 2, :],
                rhs=whh[:, kp], start=False, stop=(kp == 7),
                perf_mode=DR)

    def step_post(t, G):
        tt = t % 4
        # gate order in free dim: (g', i, f, o)
        gh = work.tile([128, 512], F16, tag="gh")
        nc.scalar.activation(gh[:], G[:],
                             mybir.ActivationFunctionType.Sigmoid,
                             scale=1.0 / SEL)
        warm(gh[:], 5)
        u = work.tile([128, 128], F16, tag="u")
        v = work.tile([128, 128], F16, tag="v")
        scr = work.tile([128, 1], F32, tag="scr")
        # v = f * c
        nc.vector.tensor_tensor(out=v[:], in0=gh[:, 256:384], in1=c_st[:],
                                op=ALU.mult)
        # u = (2g' - 1) * i   (= tanh(g) * i)
        nc.vector.affine_mul_reduce(
            out=u[:], accum_out=scr[:], in0=gh[:, 0:128],
            in1=gh[:, 128:256], scale=2.0, bias=-1.0)
        # c = u + v
        nc.vector.tensor_tensor(out=c_st[:], in0=u[:], in1=v[:], op=ALU.add)
        # tc = sigmoid(2c)  (= (tanh(c)+1)/2)
        tc_t = work.tile([128, 128], F16, tag="tc")
        nc.scalar.activation(tc_t[:], c_st[:],
                             mybir.ActivationFunctionType.Sigmoid, scale=2.0)
        warm(tc_t[:], 3)
        # h/2 = (tc - 0.5) * o
        h_t = work.tile([128, 128], BF16, tag="ht")
        scr2 = work.tile([128, 1], F32, tag="scr2")
        nc.vector.affine_mul_reduce(
            out=h_t[:], accum_out=scr2[:], in0=tc_t[:],
            in1=gh[:, 384:512], scale=1.0, bias=-0.5)
        if "htrans" in DISABLE:
            return
        # transpose h/2 -> [j, 32c+m]; fan out to block-diag fp8 tiles
        # (vector, chain-critical) + hist (vector, off-chain)
        hp = psum_h.tile([128, 128], BF16)
        nc.tensor.matmul(hp[:], lhsT=h_t[:], rhs=ident[:],
                         is_transpose=True, start=True, stop=True)
        warm(h_t[:], 1)
        warm(xg_ring[:, 0:512], 2)
        bd_fill_vec(hbd, hp, S_H)
        nc.vector.scalar_tensor_tensor(
            out=hist[:, :, tt, :], in0=hp[:], scalar=1.0, in1=ident[:],
            op0=ALU.mult, op1=ALU.bypass)
        if "x2" in DISABLE:
            return
        # x2 partial: accumulate (h/2)^2 along free dim -> x2buf[:, t]
        nc.vector.affine_mul_reduce(
            out=sq[:], accum_out=x2buf[:, t:t + 1], in0=h_t[:],
            in1=h_t[:], scale=1.0, bias=0.0)

    def proto(g):
        pp = psum_p.tile([128, 128], F32)
        for k in range(4):
            nc.tensor.matmul(
                pp[:], lhsT=hist[:, k, :, :],
                rhs=pt[:, 128 * k:128 * (k + 1)],
                start=(k == 0), stop=(k == 3))
        po = work.tile([128, 128], F32, tag="po")
        nc.vector.scalar_tensor_tensor(
            out=po[:], in0=pp[:], scalar=1.0, in1=ident[:],
            op0=ALU.mult, op1=ALU.bypass)
        nc.sync.dma_start(xp_d[:, 128 * g:128 * (g + 1)], po[:])

    # ---- main loop --------------------------------------------------------
    # Per step the tensor queue is one contiguous burst: [rec DRs(t),
    # sel(t+1), xg/proto piece] then the isolated h-transpose(t). This
    # keeps the PE array power-state ramped.
    LOOKAHEAD = 2
    for g in range(min(LOOKAHEAD, n_gran)):
        gather(g)
        for piece in range(4):
            phase1_piece(g, piece)
    G = emit_sel(0)
    for g in range(n_gran):
        if g + LOOKAHEAD < n_gran:
            gather(g + LOOKAHEAD)
        for tt in range(4):
            t = 4 * g + tt
            step_mm(G)
            if t + 1 < 4 * n_gran:
                G_next = emit_sel(t + 1)
            else:
                G_next = None
            if g + LOOKAHEAD < n_gran:
                phase1_piece(g + LOOKAHEAD, tt)
            if tt == 0 and g > 0:
                proto(g - 1)
            step_post(t, G)
            G = G_next
    proto(n_gran - 1)
    nc.sync.dma_start(x2_d[:], x2buf[:])
    ctx.close()


_PREP_CACHE = {}


def _prep_inputs(input_ids, embed_table, w_ih_f, w_hh_f, b_ih_f, b_hh_f,
                 w_ih_b, w_hh_b, b_ih_b, b_hh_b, prototypes, n_gran=NG):
    import ml_dtypes
    bf16 = ml_dtypes.bfloat16
    fp8 = ml_dtypes.float8_e4m3
    ids = np.asarray(input_ids).astype(np.int32)
    Tloc = 4 * n_gran
    key = id(embed_table)
    if key in _PREP_CACHE:
        emb = _PREP_CACHE[key]
    else:
        emb = np.ascontiguousarray(np.asarray(embed_table, np.float32)).astype(bf16)
        _PREP_CACHE[key] = emb
    prot = np.asarray(prototypes, np.float32)
    sel = _make_sel4().astype(bf16)
    per_dir = {}
    for d, (wi, wh, bi, bh) in enumerate([
            (w_ih_f, w_hh_f, b_ih_f, b_hh_f),
            (w_ih_b, w_hh_b, b_ih_b, b_hh_b)]):
        wih_bd = _bd_w(_arrange_w(np.asarray(wi, np.float32), True) * S_W)
        whh_bd = _bd_w(_arrange_w(np.asarray(wh, np.float32), True)
                       * (2.0 * S_W))
        per_dir[d] = dict(
            wih=wih_bd.astype(fp8),
            whh=whh_bd.astype(fp8),
            bb=_arrange_b(np.asarray(bi, np.float32)
                          + np.asarray(bh, np.float32)),
            pt=np.ascontiguousarray(
                prot[:, 512 * d:512 * (d + 1)].T.reshape(4, 128, 128)
            ).astype(bf16),
        )
    in_maps = []
    for core in range(8):
        d, shard = core // 4, core % 4
        ids_s = ids[8 * shard:8 * shard + 8, :Tloc]
        if d == 1:
            ids_s = ids_s[:, ::-1]
        in_maps.append(dict(
            emb=emb,
            idx=_arrange_idx(np.ascontiguousarray(ids_s), n_gran),
            wih=per_dir[d]["wih"], whh=per_dir[d]["whh"],
            bb=per_dir[d]["bb"], pt=per_dir[d]["pt"],
            sel=sel,
        ))
    return in_maps


def _combine(results, prototypes, n_gran=NG):
    Tloc = 4 * n_gran
    p2 = (np.asarray(prototypes, np.float32) ** 2).sum(-1)  # (128,)
    out = np.zeros((32, Tloc, 128), np.float32)
    for core in range(8):
        d, shard = core // 4, core % 4
        # xp rows 32q+m = (step 4g+q, row-slot m); cols = protos
        xp_raw = results[core]["xp"].reshape(4, 32, n_gran, 128)[:, 0:8]
        xp = np.transpose(xp_raw, (1, 2, 0, 3)).reshape(8, Tloc, 128)
        x2 = results[core]["x2"]                # (128, T), from (h/2)^2
        x2b = x2.reshape(4, 32, Tloc)[:, 0:8, :].sum(0)  # (8, T)
        if d == 1:
            xp = xp[:, ::-1, :]
            x2b = x2b[:, ::-1]
        sl = slice(8 * shard, 8 * shard + 8)
        out[sl] += 4.0 * xp - 4.0 * x2b[:, :, None]
    out -= p2[None, None, :]
    return out


_NC_CACHE = {}


def kernel(input_ids, embed_table, w_ih_f, w_hh_f, b_ih_f, b_hh_f,
           w_ih_b, w_hh_b, b_ih_b, b_hh_b, prototypes):
    n_gran = NG
    if n_gran not in _NC_CACHE:
        _NC_CACHE[n_gran] = build_program(n_gran)
    nc = _NC_CACHE[n_gran]
    in_maps = _prep_inputs(input_ids, embed_table, w_ih_f, w_hh_f, b_ih_f,
                           b_hh_f, w_ih_b, w_hh_b, b_ih_b, b_hh_b, prototypes,
                           n_gran)
    res = run_bass_kernel_spmd(nc, in_maps, list(range(8)))
    return _combine(res.results, prototypes, n_gran)


if __name__ == "__main__":
    import time
    t0 = time.time()
    ng = int(sys.argv[1]) if len(sys.argv) > 1 else 8
    nc = build_program(ng)
    print(f"built n_gran={ng} in {time.time()-t0:.1f}s")


# revision 51
# speedup vs baseline: 1.0223x; 1.0223x over previous
"""BiLSTM + prototype-distance kernel for 8 trn2 NeuronCores.

Sharding: 8 cores = 4 batch-shards (8 rows each) x 2 directions.
Backward-direction cores receive time-reversed input ids and run the
identical SPMD program. Host combines per-core partial outputs:
    out = 4*(xp_f + xp_b) - 4*(x2_f + x2_b) - ||protos||^2
(xp/x2 are computed from h/2 on device; the 4x compensates).

Numerics / dataflow:
- Both GEMMs (xg = e @ w_ih and the recurrent h @ w_hh) run as fp8e4
  DoubleRow matmuls (2 K-tiles per instruction, 0.5 cycles/row).
  fp8 DoubleRow can only write PSUM partition 0, so the [128,512]
  gate layout (partition = 4 hidden-chunks x 32 rows) is produced via
  a block-diagonal stationary operand: the contraction is extended to
  K=(chunk,u)=2048 and h (resp. embT) is laid out block-diagonally
  (zeros are exact in fp8). 8 accumulating DoubleRow matmuls then
  yield all four 32-partition chunk blocks in one PSUM tile.
- Quantization scales: h/2 x64, w_hh x(2*64), embT x8, w_ih x64; the
  xg path is compensated in the bias add (x 1/512), the recurrent
  path by injecting xg through a 0/4096 selector matmul and applying
  sigmoid(G/4096).
- tanh(x) = 2*sigmoid(2x)-1 throughout, so the scalar engine never
  swaps activation tables.
"""

import sys
import numpy as np

sys.path.insert(0, "/opt/trn_rl_repo")

import concourse.bass as bass  # noqa: E402
import concourse.tile as tile  # noqa: E402
import concourse.mybir as mybir  # noqa: E402
from concourse import bacc  # noqa: E402
from concourse.bass_utils import run_bass_kernel_spmd  # noqa: E402

F32 = mybir.dt.float32
F16 = mybir.dt.float16
BF16 = mybir.dt.bfloat16
FP8 = mybir.dt.float8e4
I32 = mybir.dt.int32
DR = mybir.MatmulPerfMode.DoubleRow
ALU = mybir.AluOpType

V, E, HD, P = 50000, 512, 1024, 128
H2 = HD // 2          # 512 per-direction hidden
B, T = 32, 512
BS = 8                # batch rows per core
NG = T // 4           # granules (4 timesteps each)
GMAP = [2, 0, 1, 3]   # our gate order (g, i, f, o) -> pytorch row-block order
S_W = 64.0            # fp8 scale on w_hh / w_ih
S_H = 64.0            # fp8 scale on h/2
S_E = 8.0             # fp8 scale on embeddings
SEL = S_W * S_H       # selector magnitude; sigmoid scale = 1/SEL
DISABLE = set()       # debug: subsystem names to strip from the program


def _arrange_w(w, scale_g):
    """w: (2048, K) -> (4, 128, 2048) tiles: arr[k][kk, 512c+128g+j] =
    w[512*GMAP[g] + 128c + j, 128k + kk] (*2 on the tanh gate)."""
    K = w.shape[1]
    w4 = w.reshape(4, H2, K)[GMAP].copy()      # (gamma, 512, K)
    if scale_g:
        w4[0] *= 2.0
    # -> [gamma, c, j, k, kk]
    w5 = w4.reshape(4, 4, 128, K // 128, 128)
    # arr[k, kk, c, gamma, j]
    arr = np.transpose(w5, (3, 4, 1, 0, 2)).reshape(K // 128, 128, 2048)
    return np.ascontiguousarray(arr, dtype=np.float32)


def _bd_w(arr):
    """arr: (4, 128, 2048) -> block-diag rhs (128, 8, 2, 512):
    bd[kk, kp, i, n] = arr[(2kp+i)%4][kk, 512*((2kp+i)//4) + n]."""
    bd = np.empty((128, 16, 512), np.float32)
    for kb in range(16):
        bd[:, kb, :] = arr[kb % 4][:, 512 * (kb // 4):512 * (kb // 4) + 512]
    return np.ascontiguousarray(bd.reshape(128, 8, 2, 512))


def _arrange_b(b_total):
    b4 = b_total.reshape(4, H2)[GMAP].copy()
    b4[0] *= 2.0
    # b_arr[512c + 128gamma + j] = b4[gamma, 128c + j]
    arr = np.transpose(b4.reshape(4, 4, 128), (1, 0, 2)).reshape(4, 512)
    bb = np.zeros((128, 512), np.float32)
    for c in range(4):
        bb[32 * c:32 * c + 32, :] = arr[c][None, :]
    return bb


def _make_sel4():
    """(128, 4*128): sel[p, 128*tt + q] = SEL iff q and p in the same
    32-block and p%32 == q%32 + 8*tt  (xg row selector, one matmul)."""
    sel = np.zeros((128, 4, 128), np.float32)
    for tt in range(4):
        for c in range(4):
            for m in range(32 - 8 * tt):
                sel[32 * c + 8 * tt + m, tt, 32 * c + m] = SEL
    return sel.reshape(128, 512)


def _arrange_idx(ids_shard, n_gran):
    """ids_shard: (8, T) -> (32, n_gran) int32: [8*tt + b, g] = ids[b, 4g+tt]."""
    idx = np.zeros((32, n_gran), np.int32)
    for g in range(n_gran):
        for tt in range(4):
            for b in range(BS):
                idx[8 * tt + b, g] = ids_shard[b, 4 * g + tt]
    return idx


def build_program(n_gran=NG):
    """Build the SPMD program (one core's view)."""
    nc = bacc.Bacc("TRN2", target_bir_lowering=False, debug=False)

    emb = nc.dram_tensor("emb", [V, E], BF16, kind="ExternalInput").ap()
    idx_d = nc.dram_tensor("idx", [32, n_gran], I32, kind="ExternalInput").ap()
    wih_d = nc.dram_tensor("wih", [128, 8, 2, 512], FP8, kind="ExternalInput").ap()
    whh_d = nc.dram_tensor("whh", [128, 8, 2, 512], FP8, kind="ExternalInput").ap()
    bb_d = nc.dram_tensor("bb", [128, 512], F32, kind="ExternalInput").ap()
    pt_d = nc.dram_tensor("pt", [4, 128, 128], BF16, kind="ExternalInput").ap()
    sel_d = nc.dram_tensor("sel", [128, 512], BF16, kind="ExternalInput").ap()

    xp_d = nc.dram_tensor("xp", [128, n_gran * 128], F32, kind="ExternalOutput").ap()
    x2_d = nc.dram_tensor("x2", [128, 4 * n_gran], F32, kind="ExternalOutput").ap()

    with tile.TileContext(nc) as tc:
        _body(tc, n_gran, emb, idx_d, wih_d, whh_d, bb_d, pt_d, sel_d,
              xp_d, x2_d)

    nc.compile()
    return nc


def _body(tc, n_gran, emb, idx_d, wih_d, whh_d, bb_d, pt_d, sel_d, xp_d, x2_d):
    nc = tc.nc
    from contextlib import ExitStack
    ctx = ExitStack()
    const = ctx.enter_context(tc.tile_pool(name="const", bufs=1))
    state = ctx.enter_context(tc.tile_pool(name="state", bufs=1))
    work = ctx.enter_context(tc.tile_pool(name="work", bufs=2))
    psum_g = ctx.enter_context(tc.tile_pool(name="psg", bufs=2, space="PSUM"))
    psum_m = ctx.enter_context(tc.tile_pool(name="psm", bufs=2, space="PSUM"))
    psum_t = ctx.enter_context(tc.tile_pool(name="pst", bufs=1, space="PSUM"))
    psum_h = ctx.enter_context(tc.tile_pool(name="psh", bufs=1, space="PSUM"))
    psum_p = ctx.enter_context(tc.tile_pool(name="psp", bufs=1, space="PSUM"))
    psum_w = ctx.enter_context(tc.tile_pool(name="psw", bufs=1, space="PSUM"))

    # ---- resident tensors -------------------------------------------------
    wih = const.tile([128, 8, 2, 512], FP8)
    whh = const.tile([128, 8, 2, 512], FP8)
    bb = const.tile([128, 512], F32)
    pt = const.tile([128, 4 * 128], BF16)
    sel = const.tile([128, 512], BF16)
    idx = const.tile([32, n_gran], I32)
    ident = const.tile([128, 128], BF16)

    nc.sync.dma_start(wih[:], wih_d[:])
    nc.sync.dma_start(whh[:], whh_d[:])
    for k in range(4):
        nc.sync.dma_start(pt[:, 128 * k:128 * (k + 1)], pt_d[k])
    nc.sync.dma_start(bb[:], bb_d[:])
    nc.sync.dma_start(sel[:], sel_d[:])
    nc.sync.dma_start(idx[:], idx_d[:])

    from concourse.masks import make_identity
    make_identity(nc, ident[:])

    # state
    c_st = state.tile([128, 128], F16)
    # block-diagonal fp8 stationary tiles: hbd[a][kk, b, 32a+m] = S_H*h/2
    hbd = [state.tile([128, 4, 128], FP8, name=f"hbd{a}") for a in range(4)]
    ebd = [state.tile([128, 4, 128], FP8, name=f"ebd{a}") for a in range(4)]
    hist = state.tile([128, 4, 4, 32], BF16)     # [j, c-chunk, tt, m] = h/2
    emb_ring = state.tile([32, 4 * 512], BF16)   # slot = g%4
    xg_ring = state.tile([128, 4 * 512], BF16)   # slot = g%4
    x2buf = state.tile([128, 4 * n_gran], F32)
    sq = state.tile([128, 128], F16)

    nc.gpsimd.memset(c_st[:], 0.0)
    for a in range(4):
        nc.gpsimd.memset(hbd[a][:], 0.0)
        nc.gpsimd.memset(ebd[a][:], 0.0)
    nc.gpsimd.memset(hist[:], 0.0)
    nc.gpsimd.memset(x2buf[:], 0.0)
    nc.gpsimd.memset(xg_ring[:], 0.0)
    nc.gpsimd.memset(emb_ring[:], 0.0)
    nc.gpsimd.memset(sq[:], 0.0)

    def gather(g):
        s = 512 * (g % 4)
        nc.gpsimd.indirect_dma_start(
            out=emb_ring[:, s:s + 512],
            out_offset=None,
            in_=emb[:],
            in_offset=bass.IndirectOffsetOnAxis(ap=idx[:, g:g + 1], axis=0),
        )

    def bd_fill_gp(dst, src_sb):
        """Write the 4 block-diagonal copies dst[a][:, b, 32a+m] =
        src[:, 32b+m] from an SBUF fp8 staging tile (gpsimd copies)."""
        for a in range(4):
            nc.gpsimd.affine_select(
                out=dst[a][:, :, 32 * a:32 * a + 32], in_=src_sb[:],
                pattern=[[0, 4], [0, 32]], compare_op=ALU.is_equal,
                fill=0.0, base=0, channel_multiplier=0)

    def bd_fill_vec(dst, src_psum, scale):
        """Same, directly from PSUM on the vector engine (with scale)."""
        for a in range(4):
            nc.vector.scalar_tensor_tensor(
                out=dst[a][:, :, 32 * a:32 * a + 32], in0=src_psum[:],
                scalar=scale, in1=ident[:], op0=ALU.mult, op1=ALU.bypass)

    # phase1 split into per-step pieces to avoid head-of-line blocking of
    # the serial recurrence in the engine queues.
    def phase1_piece(g, piece):
        s = 512 * (g % 4)
        if piece == 0:
            tp = psum_t.tile([128, 128], BF16)
            for k in range(4):
                nc.tensor.matmul(
                    tp[:, 32 * k:32 * k + 32],
                    lhsT=emb_ring[:, s + 128 * k:s + 128 * (k + 1)],
                    rhs=ident[:32, :32],
                    is_transpose=True, start=(k == 0), stop=(k == 3))
            tq = work.tile([128, 128], FP8, tag="tq")
            nc.vector.scalar_tensor_tensor(
                out=tq[:], in0=tp[:], scalar=S_E, in1=ident[:],
                op0=ALU.mult, op1=ALU.bypass)
            bd_fill_gp(ebd, tq)
        else:
            if piece == 1:
                phase1_piece.mm = psum_m.tile([128, 512], F32)
            mm = phase1_piece.mm
            for kp in range([0, 0, 2, 5][piece], [0, 2, 5, 8][piece]):
                a, q = kp // 2, kp % 2
                nc.tensor.matmul(
                    mm[:], lhsT=ebd[a][:, 2 * q:2 * q + 2, :],
                    rhs=wih[:, kp], start=(kp == 0), stop=(kp == 7),
                    perf_mode=DR)
            if piece == 3:
                for half in range(2):
                    sl = slice(s + 256 * half, s + 256 * (half + 1))
                    nc.vector.scalar_tensor_tensor(
                        out=xg_ring[:, sl],
                        in0=mm[:, 256 * half:256 * (half + 1)],
                        scalar=1.0 / (S_E * S_W),
                        in1=bb[:, 256 * half:256 * (half + 1)],
                        op0=ALU.mult, op1=ALU.add)

    warm_tile = [None]

    def warm(rhs, n=1):
        """Dummy matmul into a scratch PSUM bank: keeps the PE power-state
        ramped through the postproc gap (result unused). Passing a
        mid-chain tensor as rhs delays execution into the gap."""
        if warm_tile[0] is None:
            warm_tile[0] = psum_w.tile([128, 512], F32, name="warmt")
        w = rhs.free_size()
        for _ in range(n):
            nc.tensor.matmul(
                warm_tile[0][:, 0:w], lhsT=sel[:, 0:128], rhs=rhs,
                start=True, stop=True, skip_group_check=True)

    def emit_sel(t):
        """Preload the xg injection for step t into a fresh PSUM buffer:
        one K=128 matmul, sel = 0/4096 shift matrix."""
        tt, slot = t % 4, 512 * ((t // 4) % 4)
        G = psum_g.tile([128, 512], F32)
        nc.tensor.matmul(
            G[:], lhsT=sel[:, 128 * tt:128 * (tt + 1)],
            rhs=xg_ring[:, slot:slot + 512],
            start=True, stop=False)
        return G

    def step_mm(G):
        # recurrent part: 8 fp8 DoubleRow matmuls, block-diag stationary
        for kp in range(8):
            a, q = kp // 2, kp % 2
            nc.tensor.matmul(
                G[:], lhsT=hbd[a][:, 2 * q:2 * q + 2, :],
                rhs=whh[:, kp], start=False, stop=(kp == 7),
                perf_mode=DR)

    def step_post(t, G):
        tt = t % 4
        # gate order in free dim: (g', i, f, o)
        gh = work.tile([128, 512], F16, tag="gh")
        nc.scalar.activation(gh[:], G[:],
                             mybir.ActivationFunctionType.Sigmoid,
                             scale=1.0 / SEL)
        warm(gh[:], 5)
        u = work.tile([128, 128], F16, tag="u")
        v = work.tile([128, 128], F16, tag="v")
        scr = work.tile([128, 1], F32, tag="scr")
        # v = f * c
        nc.vector.tensor_tensor(out=v[:], in0=gh[:, 256:384], in1=c_st[:],
                                op=ALU.mult)
        # u = (2g' - 1) * i   (= tanh(g) * i)
        nc.vector.affine_mul_reduce(
            out=u[:], accum_out=scr[:], in0=gh[:, 0:128],
            in1=gh[:, 128:256], scale=2.0, bias=-1.0)
        # c = u + v
        nc.vector.tensor_tensor(out=c_st[:], in0=u[:], in1=v[:], op=ALU.add)
        # tc = sigmoid(2c)  (= (tanh(c)+1)/2)
        tc_t = work.tile([128, 128], F16, tag="tc")
        nc.scalar.activation(tc_t[:], c_st[:],
                             mybir.ActivationFunctionType.Sigmoid, scale=2.0)
        # h/2 = (tc - 0.5) * o
        h_t = work.tile([128, 128], BF16, tag="ht")
        scr2 = work.tile([128, 1], F32, tag="scr2")
        nc.vector.affine_mul_reduce(
            out=h_t[:], accum_out=scr2[:], in0=tc_t[:],
            in1=gh[:, 384:512], scale=1.0, bias=-0.5)
        if "htrans" in DISABLE:
            return
        # transpose h/2 -> [j, 32c+m]; fan out to block-diag fp8 tiles
        # (vector, chain-critical) + hist (vector, off-chain)
        hp = psum_h.tile([128, 128], BF16)
        nc.tensor.matmul(hp[:], lhsT=h_t[:], rhs=ident[:],
                         is_transpose=True, start=True, stop=True)
        warm(h_t[:], 1)
        warm(xg_ring[:, 0:512], 2)
        bd_fill_vec(hbd, hp, S_H)
        nc.vector.scalar_tensor_tensor(
            out=hist[:, :, tt, :], in0=hp[:], scalar=1.0, in1=ident[:],
            op0=ALU.mult, op1=ALU.bypass)
        if "x2" in DISABLE:
            return
        # x2 partial: accumulate (h/2)^2 along free dim -> x2buf[:, t]
        nc.vector.affine_mul_reduce(
            out=sq[:], accum_out=x2buf[:, t:t + 1], in0=h_t[:],
            in1=h_t[:], scale=1.0, bias=0.0)

    def proto(g):
        pp = psum_p.tile([128, 128], F32)
        for k in range(4):
            nc.tensor.matmul(
                pp[:], lhsT=hist[:, k, :, :],
                rhs=pt[:, 128 * k:128 * (k + 1)],
                start=(k == 0), stop=(k == 3))
        po = work.tile([128, 128], F32, tag="po")
        nc.vector.scalar_tensor_tensor(
            out=po[:], in0=pp[:], scalar=1.0, in1=ident[:],
            op0=ALU.mult, op1=ALU.bypass)
        nc.sync.dma_start(xp_d[:, 128 * g:128 * (g + 1)], po[:])

    # ---- main loop --------------------------------------------------------
    # Per step the tensor queue is one contiguous burst: [rec DRs(t),
    # sel(t+1), xg/proto piece] then the isolated h-transpose(t). This
    # keeps the PE array power-state ramped.
    LOOKAHEAD = 2
    for g in range(min(LOOKAHEAD, n_gran)):
        gather(g)
        for piece in range(4):
            phase1_piece(g, piece)
    G = emit_sel(0)
    for g in range(n_gran):
        if g + LOOKAHEAD < n_gran:
            gather(g + LOOKAHEAD)
        for tt in range(4):
            t = 4 * g + tt
            step_mm(G)
            if t + 1 < 4 * n_gran:
                G_next = emit_sel(t + 1)
            else:
                G_next = None
            if g + LOOKAHEAD < n_gran:
                phase1_piece(g + LOOKAHEAD, tt)
            if tt == 0 and g > 0:
                proto(g - 1)
            step_post(t, G)
            G = G_next
    proto(n_gran - 1)
    nc.sync.dma_start(x2_d[:], x2buf[:])
    ctx.close()


_PREP_CACHE = {}


def _prep_inputs(input_ids, embed_table, w_ih_f, w_hh_f, b_ih_f, b_hh_f,
                 w_ih_b, w_hh_b, b_ih_b, b_hh_b, prototypes, n_gran=NG):
    import ml_dtypes
    bf16 = ml_dtypes.bfloat16
    fp8 = ml_dtypes.float8_e4m3
    ids = np.asarray(input_ids).astype(np.int32)
    Tloc = 4 * n_gran
    key = id(embed_table)
    if key in _PREP_CACHE:
        emb = _PREP_CACHE[key]
    else:
        emb = np.ascontiguousarray(np.asarray(embed_table, np.float32)).astype(bf16)
        _PREP_CACHE[key] = emb
    prot = np.asarray(prototypes, np.float32)
    sel = _make_sel4().astype(bf16)
    per_dir = {}
    for d, (wi, wh, bi, bh) in enumerate([
            (w_ih_f, w_hh_f, b_ih_f, b_hh_f),
            (w_ih_b, w_hh_b, b_ih_b, b_hh_b)]):
        wih_bd = _bd_w(_arrange_w(np.asarray(wi, np.float32), True) * S_W)
        whh_bd = _bd_w(_arrange_w(np.asarray(wh, np.float32), True)
                       * (2.0 * S_W))
        per_dir[d] = dict(
            wih=wih_bd.astype(fp8),
            whh=whh_bd.astype(fp8),
            bb=_arrange_b(np.asarray(bi, np.float32)
                          + np.asarray(bh, np.float32)),
            pt=np.ascontiguousarray(
                prot[:, 512 * d:512 * (d + 1)].T.reshape(4, 128, 128)
            ).astype(bf16),
        )
    in_maps = []
    for core in range(8):
        d, shard = core // 4, core % 4
        ids_s = ids[8 * shard:8 * shard + 8, :Tloc]
        if d == 1:
            ids_s = ids_s[:, ::-1]
        in_maps.append(dict(
            emb=emb,
            idx=_arrange_idx(np.ascontiguousarray(ids_s), n_gran),
            wih=per_dir[d]["wih"], whh=per_dir[d]["whh"],
            bb=per_dir[d]["bb"], pt=per_dir[d]["pt"],
            sel=sel,
        ))
    return in_maps


def _combine(results, prototypes, n_gran=NG):
    Tloc = 4 * n_gran
    p2 = (np.asarray(prototypes, np.float32) ** 2).sum(-1)  # (128,)
    out = np.zeros((32, Tloc, 128), np.float32)
    for core in range(8):
        d, shard = core // 4, core % 4
        # xp rows 32q+m = (step 4g+q, row-slot m); cols = protos
        xp_raw = results[core]["xp"].reshape(4, 32, n_gran, 128)[:, 0:8]
        xp = np.transpose(xp_raw, (1, 2, 0, 3)).reshape(8, Tloc, 128)
        x2 = results[core]["x2"]                # (128, T), from (h/2)^2
        x2b = x2.reshape(4, 32, Tloc)[:, 0:8, :].sum(0)  # (8, T)
        if d == 1:
            xp = xp[:, ::-1, :]
            x2b = x2b[:, ::-1]
        sl = slice(8 * shard, 8 * shard + 8)
        out[sl] += 4.0 * xp - 4.0 * x2b[:, :, None]
    out -= p2[None, None, :]
    return out


_NC_CACHE = {}


def kernel(input_ids, embed_table, w_ih_f, w_hh_f, b_ih_f, b_hh_f,
           w_ih_b, w_hh_b, b_ih_b, b_hh_b, prototypes):
    n_gran = NG
    if n_gran not in _NC_CACHE:
        _NC_CACHE[n_gran] = build_program(n_gran)
    nc = _NC_CACHE[n_gran]
    in_maps = _prep_inputs(input_ids, embed_table, w_ih_f, w_hh_f, b_ih_f,
                           b_hh_f, w_ih_b, w_hh_b, b_ih_b, b_hh_b, prototypes,
                           n_gran)
    res = run_bass_kernel_spmd(nc, in_maps, list(range(8)))
    return _combine(res.results, prototypes, n_gran)


if __name__ == "__main__":
    import time
    t0 = time.time()
    ng = int(sys.argv[1]) if len(sys.argv) > 1 else 8
    nc = build_program(ng)
    print(f"built n_gran={ng} in {time.time()-t0:.1f}s")
